# revision 19
# baseline (speedup 1.0000x reference)
"""CrossAttention Trainium2 kernel: B=4, S=2048, H=1024, NH=16, HD=64.

Sharding: 8 cores = (batch b in 0..3) x (head-group g in 0..1).
Core c=2b+g computes batch b, heads [8g, 8g+8) end-to-end (Q/K/V projection,
causal flash attention in transposed-score layout, output projection over its
512 Wo rows), producing a partial [2048,1024] output; host sums the g=0/g=1
partials per batch (row-parallel Wo reduction) and stacks batches.

Device layout notes:
 - Projections/PV/out-projection in bf16 (PSUM fp32); host pre-transposes
   x,y to xT/yT [H,S] so every contraction dim lands in the partition dim.
 - Scores run in fp8e4m3 with MatmulPerfMode.DoubleRow: Q/K projections are
   copied PSUM->SBUF as fp8, then partition-shift SBUF->SBUF DMAs repack
   each head's [64,S] into [32, 2*S] (d-halves side by side in the free
   dim), so one DoubleRow matmul contracts both 32-wide d-halves at 0.5
   cycles/row - 2x the bf16 score rate.
 - Scores computed transposed, S^T[k,q]: lhsT=K8 [32,2,128] stationary,
   rhs=Q8 [32,2,512] moving. exp via ACT (scale=1/8 folded in), output E^T
   bf16 - no P transpose needed.
 - P@V: lhsT=V_ext [128k, 65] (col 64 = ones -> row 64 of AO^T = softmax
   denominator for free), rhs=E^T [128k, 512q], accumulated over k blocks.
 - Causal: k-blocks strictly above the diagonal are skipped; diagonal
   blocks compute only the valid q-slice, with a triangular 0/1 bf16 mask
   multiply on the one partially-valid 128x128 sub-block.
 - Normalization: recip of AO^T row 64 (DVE), broadcast across the 64 hd
   partitions via gpsimd partition_broadcast, then one DVE multiply into
   the bf16 AO^T slab used as lhsT of the out-projection.
 - Scheduling: software pipeline. Warmup projects Q/K for head-pair 0 and
   V for k-blocks 0-7; attention groups (hp, qtile) then run with the
   remaining projection / V / out-projection work queued as background PE
   units popped between k-block iterations, so the ACT exp stream starts
   ~100us earlier and the PE never drains.
"""
import sys

sys.path.insert(0, "/opt/trn_rl_repo")

import numpy as np
import ml_dtypes

import concourse.bass as bass
import concourse.tile as tile
from concourse import bacc, mybir
from concourse.bass_utils import run_bass_kernel_spmd

BF16 = ml_dtypes.bfloat16
B, S, H, NH = 4, 2048, 1024, 16
HD = H // NH  # 64
GH = NH // 2  # heads per core = 8
GW = GH * HD  # per-core projection width = 512

LAST_RESULT = None  # test harness reads exec_time_ns from here

_CACHE = {}


def _build(with_bias_q, with_bias_k, with_bias_v, with_kp):
    key = (with_bias_q, with_bias_k, with_bias_v, with_kp)
    if key in _CACHE:
        return _CACHE[key]

    f32 = mybir.dt.float32
    bf16 = mybir.dt.bfloat16
    fp8 = mybir.dt.float8e4
    DR = mybir.MatmulPerfMode.DoubleRow

    nc = bacc.Bacc("TRN2", target_bir_lowering=False, debug=False)

    xT = nc.dram_tensor("xT", [H, S], bf16, kind="ExternalInput")
    yT = nc.dram_tensor("yT", [H, S], bf16, kind="ExternalInput")
    wq = nc.dram_tensor("wq", [H, GW], bf16, kind="ExternalInput")
    wk = nc.dram_tensor("wk", [H, GW], bf16, kind="ExternalInput")
    wv = nc.dram_tensor("wv", [H, GW], bf16, kind="ExternalInput")
    wo = nc.dram_tensor("wo", [GW, H], bf16, kind="ExternalInput")
    tri = nc.dram_tensor("tri", [128, 128], bf16, kind="ExternalInput")
    if with_bias_q:
        bq = nc.dram_tensor("bq", [128, 4], f32, kind="ExternalInput")
    if with_bias_k:
        bk = nc.dram_tensor("bk", [128, 4], f32, kind="ExternalInput")
    if with_bias_v:
        bv = nc.dram_tensor("bv", [128, GW], f32, kind="ExternalInput")
    if with_kp:
        kp = nc.dram_tensor("kp", [128, 16], f32, kind="ExternalInput")
    out = nc.dram_tensor("out", [S, H], f32, kind="ExternalOutput")

    NKB = S // 128  # 16 k blocks
    VST = HD + 1  # v stripe width 65
    QW = 1024

    with tile.TileContext(nc) as tc:
        from contextlib import ExitStack

        with ExitStack() as ctx:
            big = ctx.enter_context(tc.tile_pool(name="big", bufs=1))
            ring = ctx.enter_context(tc.tile_pool(name="ring", bufs=2, space="PSUM"))
            aop = ctx.enter_context(tc.tile_pool(name="aop", bufs=2, space="PSUM"))
            e_pool = ctx.enter_context(tc.tile_pool(name="e", bufs=5))
            r_pool = ctx.enter_context(tc.tile_pool(name="r", bufs=1))
            st_pool = ctx.enter_context(tc.tile_pool(name="st", bufs=2))
            o_pool = ctx.enter_context(tc.tile_pool(name="o", bufs=2))

            wq_sb = big.tile([128, 8 * GW], bf16, tag="wq")
            wk_sb = big.tile([128, 8 * GW], bf16, tag="wk")
            wv_sb = big.tile([128, 8 * GW], bf16, tag="wv")
            wo_sb = big.tile([128, 4 * H], bf16, tag="wo")
            xT_sb = big.tile([128, 8 * S], bf16, tag="xT")
            yT_sb = big.tile([128, 8 * S], bf16, tag="yT")
            qf8 = big.tile([128, 4 * S], fp8, tag="qf8")
            kf8 = big.tile([128, 4 * S], fp8, tag="kf8")
            # DoubleRow layouts: one tile per head-pair; row sub*32 + p is
            # d-dim (32j+p) of head 2*hp+sub at column block j*S.
            q8 = [big.tile([64, 2 * S], fp8, tag=f"q8{t}", name=f"q8{t}") for t in range(4)]
            k8 = [big.tile([64, 2 * S], fp8, tag=f"k8{t}", name=f"k8{t}") for t in range(4)]
            v_sb = big.tile([128, NKB * GH * VST], bf16, tag="v")
            ao_sb = big.tile([128, 4 * S], bf16, tag="ao")
            tri_sb = big.tile([128, 128], bf16, tag="tri")

            # ---- input DMAs: Q-projection inputs first so the PE can start
            # as early as possible, then K, V, O inputs.
            for hc in range(8):
                nc.sync.dma_start(
                    wq_sb[:, hc * GW : (hc + 1) * GW],
                    wq.ap()[hc * 128 : (hc + 1) * 128, :],
                )
            for hc in range(8):
                nc.sync.dma_start(
                    xT_sb[:, hc * S : (hc + 1) * S],
                    xT.ap()[hc * 128 : (hc + 1) * 128, :],
                )
            for hc in range(8):
                nc.sync.dma_start(
                    wk_sb[:, hc * GW : (hc + 1) * GW],
                    wk.ap()[hc * 128 : (hc + 1) * 128, :],
                )
            for hc in range(8):
                nc.sync.dma_start(
                    yT_sb[:, hc * S : (hc + 1) * S],
                    yT.ap()[hc * 128 : (hc + 1) * 128, :],
                )
            for hc in range(8):
                nc.sync.dma_start(
                    wv_sb[:, hc * GW : (hc + 1) * GW],
                    wv.ap()[hc * 128 : (hc + 1) * 128, :],
                )
            for hp in range(4):
                nc.sync.dma_start(
                    wo_sb[:, hp * H : (hp + 1) * H],
                    wo.ap()[hp * 128 : (hp + 1) * 128, :],
                )
            nc.sync.dma_start(tri_sb[:], tri.ap()[:])

            bias_tiles = {}
            if with_bias_q:
                bias_tiles["bq"] = big.tile([128, 4], f32, tag="bq")
                nc.gpsimd.dma_start(bias_tiles["bq"][:], bq.ap()[:])
            if with_bias_k:
                bias_tiles["bk"] = big.tile([128, 4], f32, tag="bk")
                nc.gpsimd.dma_start(bias_tiles["bk"][:], bk.ap()[:])
            if with_bias_v:
                bias_tiles["bv"] = big.tile([128, GW], f32, tag="bv")
                nc.gpsimd.dma_start(bias_tiles["bv"][:], bv.ap()[:])
            if with_kp:
                bias_tiles["kp"] = big.tile([128, 16], f32, tag="kp")
                nc.gpsimd.dma_start(bias_tiles["kp"][:], kp.ap()[:])

            vst_view = v_sb[:].rearrange("p (n m) -> p n m", m=VST)
            nc.vector.memset(vst_view[:, :, HD : HD + 1], 1.0)

            # ---- PE work units -------------------------------------------
            def qk_unit(name, hp, st):
                # one 512-col chunk of the Q or K projection for head-pair hp
                w_sb = wq_sb if name == "q" else wk_sb
                src = xT_sb if name == "q" else yT_sb
                dst = qf8 if name == "q" else kf8
                bias_key = "bq" if name == "q" else "bk"
                ps = ring.tile([128, 512], f32, tag="mm", name=f"p{name}{hp}{st}")
                for hc in range(8):
                    nc.tensor.matmul(
                        ps[:],
                        w_sb[:, hc * GW + hp * 128 : hc * GW + hp * 128 + 128],
                        src[:, hc * S + st * 512 : hc * S + st * 512 + 512],
                        start=(hc == 0),
                        stop=(hc == 7),
                    )
                if bias_key in bias_tiles:
                    nc.vector.tensor_scalar_add(
                        ps[:], ps[:], bias_tiles[bias_key][:, hp : hp + 1]
                    )
                nc.vector.tensor_copy(
                    dst[:, hp * S + st * 512 : hp * S + st * 512 + 512], ps[:]
                )

            def rearr_unit(name, hp):
                # partition-shift repack [64,S] -> [32, 2*S] for DoubleRow
                src = qf8 if name == "q" else kf8
                dst = (q8 if name == "q" else k8)[hp]
                for sub in range(2):
                    for j in range(2):
                        nc.sync.dma_start(
                            dst[sub * 32 : sub * 32 + 32, j * S : (j + 1) * S],
                            src[sub * 64 + 32 * j : sub * 64 + 32 * j + 32,
                                hp * S : (hp + 1) * S],
                        )

            def v_unit(kb):
                ps = ring.tile([128, 512], f32, tag="mm", name=f"pv{kb}")
                for hc in range(8):
                    nc.tensor.matmul(
                        ps[:],
                        yT_sb[:, hc * S + kb * 128 : hc * S + kb * 128 + 128],
                        wv_sb[:, hc * GW : (hc + 1) * GW],
                        start=(hc == 0),
                        stop=(hc == 7),
                    )
                if "bv" in bias_tiles:
                    nc.vector.tensor_add(ps[:], ps[:], bias_tiles["bv"][:])
                for h in range(GH):
                    nc.vector.tensor_copy(
                        v_sb[:, (kb * GH + h) * VST : (kb * GH + h) * VST + HD],
                        ps[:, h * HD : (h + 1) * HD],
                    )
                if "kp" in bias_tiles:
                    nc.vector.tensor_scalar_mul(
                        v_sb[:, kb * GH * VST : (kb + 1) * GH * VST],
                        v_sb[:, kb * GH * VST : (kb + 1) * GH * VST],
                        bias_tiles["kp"][:, kb : kb + 1],
                    )

            def op_unit(qb, oc):
                # half of the output projection for 128 rows
                ps = ring.tile([128, 512], f32, tag="mm", name=f"op{qb}{oc}")
                for hp2 in range(4):
                    nc.tensor.matmul(
                        ps[:],
                        ao_sb[:, hp2 * S + qb * 128 : hp2 * S + qb * 128 + 128],
                        wo_sb[:, hp2 * H + oc * 512 : hp2 * H + oc * 512 + 512],
                        start=(hp2 == 0),
                        stop=(hp2 == 3),
                    )
                osb = o_pool.tile([128, 512], f32, tag="o")
                nc.vector.tensor_copy(osb[:], ps[:])
                nc.sync.dma_start(
                    out.ap()[qb * 128 : qb * 128 + 128, oc * 512 : oc * 512 + 512],
                    osb[:],
                )

            # ---- background queue ----------------------------------------
            bg = []  # (tag, cost_ns, thunk)

            credit = [0]

            def pop_bg(budget):
                # accumulate credit and pop PSUM-using units in pairs (ring
                # phase parity) only when enough ACT-slack has built up, so
                # background work is spread over the whole attention phase
                credit[0] += budget
                while len(bg) >= 2 and credit[0] >= bg[0][1] + bg[1][1]:
                    for _ in range(2):
                        tag, cost, fn = bg.pop(0)
                        fn()
                        credit[0] -= cost

            def drain_bg(tags):
                taken = 0
                i = 0
                while i < len(bg):
                    if bg[i][0] in tags:
                        _, _, fn = bg.pop(i)
                        fn()
                        taken += 1
                    else:
                        i += 1
                if taken % 2 and bg:
                    _, _, fn = bg.pop(0)
                    fn()

            # ---- attention group -----------------------------------------
            def attention(hp, qt):
                n_kb = (qt + 1) * (QW // 128)
                d0 = qt * (QW // 128)
                aos = {}
                for sub in range(2):
                    aos[sub] = aop.tile(
                        [128, QW], f32, tag="ao", name=f"ao{hp}{qt}{sub}"
                    )
                q8t, k8t = q8[hp], k8[hp]
                kb_order = [d0] + list(range(n_kb - 1, d0, -1)) + list(range(d0))
                pending = []  # lag-1 PV emission: (kb, ki, sub, eT)

                def emit_pv(kb, ki, sub, eT):
                    h = 2 * hp + sub
                    m = kb - d0
                    f0 = 128 * m if m > 0 else 0
                    for half in range(QW // 512):
                        h0 = half * 512
                        if h0 + 512 <= f0:
                            continue
                        s0 = max(f0, h0)
                        nc.tensor.matmul(
                            aos[sub][0:VST, s0 : h0 + 512],
                            v_sb[:, (kb * GH + h) * VST : (kb * GH + h + 1) * VST],
                            eT[:, s0 : h0 + 512],
                            start=(ki == 0),
                            stop=(ki == n_kb - 1),
                        )

                for ki, kb in enumerate(kb_order):
                    m = kb - d0
                    f0 = 128 * m if m > 0 else 0
                    c = QW - f0
                    for sub in range(2):
                        r0 = sub * 32
                        kv = k8t[r0 : r0 + 32, :].rearrange(
                            "p (two s) -> p two s", two=2
                        )
                        qv = q8t[r0 : r0 + 32, :].rearrange(
                            "p (two s) -> p two s", two=2
                        )
                        sT = ring.tile([128, QW], f32, tag="mm", name="sT")
                        for half in range(QW // 512):
                            h0 = half * 512
                            if h0 + 512 <= f0:
                                continue
                            s0 = max(f0, h0)
                            nc.tensor.matmul(
                                sT[:, s0 : h0 + 512],
                                kv[:, :, kb * 128 : kb * 128 + 128],
                                qv[:, :, qt * QW + s0 : qt * QW + h0 + 512],
                                start=True,
                                stop=True,
                                perf_mode=DR,
                            )
                        eT = e_pool.tile([128, QW], bf16, tag="e")
                        nc.scalar.activation(
                            eT[:, f0:],
                            sT[:, f0:],
                            mybir.ActivationFunctionType.Exp,
                            scale=0.125,
                        )
                        if m >= 0:
                            nc.vector.tensor_mul(
                                eT[:, f0 : f0 + 128], eT[:, f0 : f0 + 128], tri_sb[:]
                            )
                        pending.append((kb, ki, sub, eT))
                    # PVs for the previous iteration: exp for them finished
                    # long ago, so the PE never waits on ACT latency here
                    while len(pending) > 2:
                        emit_pv(*pending.pop(0))
                    pop_bg(int(0.53 * c) + 475)
                while pending:
                    emit_pv(*pending.pop(0))
                for sub in range(2):
                    po = 64 * sub
                    # free the PSUM slot with one Pool copy; normalize from
                    # the bf16 stage so the next group's PV can start
                    st = st_pool.tile([VST, QW], bf16, tag="st")
                    nc.vector.tensor_copy(st[:], aos[sub][0:VST, :])
                    rinv = r_pool.tile([1, QW], f32, tag="r")
                    nc.vector.reciprocal(rinv[:], st[HD : HD + 1, :])
                    bc_sb = r_pool.tile([HD, QW], f32, tag="bcsb")
                    nc.gpsimd.partition_broadcast(bc_sb[:], rinv[:])
                    nc.vector.tensor_mul(
                        ao_sb[po : po + 64, hp * S + qt * QW : hp * S + qt * QW + QW],
                        st[0:HD, :],
                        bc_sb[:],
                    )

            # ---- schedule ------------------------------------------------
            # warmup: Q/K for head-pair 0, V for k-blocks 0-7
            for st in range(4):
                qk_unit("q", 0, st)
            rearr_unit("q", 0)
            for st in range(4):
                qk_unit("k", 0, st)
            rearr_unit("k", 0)
            for kb in range(8):
                v_unit(kb)

            def qk_full(name, hp, st):
                # fold the rearrange DMA into the last chunk so bg holds
                # only ring-allocating units
                def fn():
                    qk_unit(name, hp, st)
                    if st == 3:
                        rearr_unit(name, hp)

                return fn

            for hp in range(1, 4):
                for name in ("q", "k"):
                    for st in range(4):
                        bg.append((f"qk{hp}", 1707, qk_full(name, hp, st)))
            for kb in range(8, 16):
                bg.append(("v2", 1707, (lambda b: lambda: v_unit(b))(kb)))

            groups = [(0, 0), (1, 0), (2, 0), (3, 0), (0, 1), (1, 1), (2, 1), (3, 1)]
            for g, (hp, qt) in enumerate(groups):
                need = {f"qk{hp}"}
                if qt == 1:
                    need.add("v2")
                drain_bg(need)
                attention(hp, qt)
                if g == 3:
                    for qb in range(8):
                        for oc in range(2):
                            bg.append(
                                ("op", 853, (lambda q, o: lambda: op_unit(q, o))(qb, oc))
                            )

            drain_bg({f"qk{h}" for h in range(4)} | {"v2", "op"})
            for qb in range(8, 16):
                for oc in range(2):
                    op_unit(qb, oc)

    nc.compile()
    _CACHE[key] = nc
    return nc


def kernel(x, y, mask, Wq_w, Wq_b, Wkv_w, Wkv_b, Wo_w, Wo_b):
    global LAST_RESULT
    x = np.asarray(x)
    y = np.asarray(y)
    mask = np.asarray(mask)
    Wq_w = np.asarray(Wq_w, dtype=np.float32)
    Wq_b = np.asarray(Wq_b, dtype=np.float32)
    Wkv_w = np.asarray(Wkv_w, dtype=np.float32)
    Wkv_b = np.asarray(Wkv_b, dtype=np.float32)
    Wo_w = np.asarray(Wo_w, dtype=np.float32)
    Wo_b = np.asarray(Wo_b, dtype=np.float32)

    with_bias_q = bool(np.any(Wq_b))
    with_bias_k = bool(np.any(Wkv_b[:H]))
    with_bias_v = bool(np.any(Wkv_b[H:]))
    with_kp = bool(np.any(mask))

    nc = _build(with_bias_q, with_bias_k, with_bias_v, with_kp)

    tri = (np.arange(128)[None, :] >= np.arange(128)[:, None]).astype(BF16)

    xT_b = [np.ascontiguousarray(x[b].astype(BF16).T) for b in range(B)]
    yT_b = [np.ascontiguousarray(y[b].astype(BF16).T) for b in range(B)]

    in_maps = []
    for c in range(8):
        b, g = c // 2, c % 2
        im = {
            "xT": xT_b[b],
            "yT": yT_b[b],
            "wq": np.ascontiguousarray(Wq_w[:, g * GW : (g + 1) * GW]).astype(BF16),
            "wk": np.ascontiguousarray(Wkv_w[:, g * GW : (g + 1) * GW]).astype(BF16),
            "wv": np.ascontiguousarray(
                Wkv_w[:, H + g * GW : H + (g + 1) * GW]
            ).astype(BF16),
            "wo": np.ascontiguousarray(Wo_w[g * GW : (g + 1) * GW, :]).astype(BF16),
            "tri": tri,
        }
        if with_bias_q:
            im["bq"] = np.ascontiguousarray(
                Wq_b[g * GW : (g + 1) * GW].reshape(4, 128).T
            ).astype(np.float32)
        if with_bias_k:
            im["bk"] = np.ascontiguousarray(
                Wkv_b[g * GW : (g + 1) * GW].reshape(4, 128).T
            ).astype(np.float32)
        if with_bias_v:
            im["bv"] = np.broadcast_to(
                Wkv_b[H + g * GW : H + (g + 1) * GW], (128, GW)
            ).astype(np.float32)
        if with_kp:
            im["kp"] = np.ascontiguousarray(
                (~mask[b]).astype(np.float32).reshape(16, 128).T
            )
        in_maps.append(im)

    LAST_RESULT = run_bass_kernel_spmd(nc, in_maps, list(range(8)))
    res = LAST_RESULT.results

    outp = np.empty((B, S, H), dtype=np.float32)
    for b in range(B):
        outp[b] = res[2 * b]["out"] + res[2 * b + 1]["out"]
    if np.any(Wo_b):
        outp += Wo_b
    return outp
